# revision 9
# baseline (speedup 1.0000x reference)
"""Multi-head attention on 8 Trainium2 NeuronCores — fp16 + fp8 scores.

Sharding: tensor-parallel over heads (2 heads/core), full batch on every
core; host sums the 8 partial outputs and adds b_o + b_v @ w_o.

vs the fp16 version: the score matmul S^T = K^T·Q runs in fp8 e4m3 with
perf_mode=DoubleRow (0.5 cycles/row): qT/kT are stored scaled x16 as
[32, 2, S] (dh split p+32i), produced directly by the projection's DVE
bias-add (w_q/w_k columns host-permuted so the PSUM partition order is
[h0 d0-31 | h1 d0-31 | h0 d32-63 | h1 d32-63]). exp scale absorbs the
1/256.

Pipeline (per batch bi):
  A: xT[kt] [128, S] fp16 DMA'd in 512-col chunks (kt-major per chunk)
     so batch-0 projections start after ~1MB instead of 4MB.
  B: QK projections (fp16 MM -> PSUM; 2 DVE ops cast+bias -> fp8 tile),
     V natural via lhsT=xT tile (fp16, unchanged).
  C: jc-OUTER (2 chunks of 1024 q), h inner; per kt: S^T fp8 MM,
     D-unit pop (rate-controlled), exp (ACT), AV fp16 MM w/ ones row.
     rline r-path: SBUF->SBUF DMA on the ACT hwdge queue (no DRAM
     roundtrip, no head-of-line behind bulk x/out DMAs), PE transpose +
     reciprocal deferred to where the DMA has had time to land.
  D: per (tt, half): poA/poB = attnT_h^T @ w_o_h (tile_position rows),
     t1 = poA*rrec0 (DVE; ACT for the tail flush), osb = poB*rrec1 + t1
     -> fp16 -> DMA out. D(bi)-jc0 units interleave into C(bi)-jc1;
     D(bi)-jc1 units into C(bi+1)-jc0; only b3-jc1 flushes at the end.
"""

import numpy as np

import concourse.bacc as bacc
import concourse.mybir as mybir
from concourse.tile import TileContext
from concourse.masks import make_identity
from concourse import bass_utils

dt = mybir.dt
F32 = dt.float32
F16 = dt.float16
F8 = dt.float8e4
AF = mybir.ActivationFunctionType
ALU = mybir.AluOpType
DR = mybir.MatmulPerfMode.DoubleRow

B, S, D = 4, 2048, 1024
H, DH = 16, 64
NCORES = 8
HPC = H // NCORES          # heads per core = 2
DHC = HPC * DH             # 128 projection cols per core
QK_SCALE = 16.0            # q,k pre-scale folded into w/b; exp scale /256

_CACHE = {}


def build_nc(b=B, s=S):
    d = D
    n_tt = s // 128            # token tiles per batch
    n_kt = d // 128            # contraction tiles for projections
    qw = 1024 if s >= 1024 else s
    n_jc = s // qw             # q chunks per batch
    ntj = qw // 128            # token tiles per q chunk
    assert s % 512 == 0 and d == 1024
    assert n_jc == 2, "C-stage interleave schedule assumes two q chunks"

    nc = bacc.Bacc("TRN2", target_bir_lowering=False, debug=False)

    x_d = nc.dram_tensor("x", [b, d, s], F16, kind="ExternalInput")
    wq_d = nc.dram_tensor("wq", [d, DHC], F16, kind="ExternalInput")
    wk_d = nc.dram_tensor("wk", [d, DHC], F16, kind="ExternalInput")
    wv_d = nc.dram_tensor("wv", [d, DHC], F16, kind="ExternalInput")
    bq_d = nc.dram_tensor("bq", [DHC, 1], F32, kind="ExternalInput")
    bk_d = nc.dram_tensor("bk", [DHC, 1], F32, kind="ExternalInput")
    wo_d = nc.dram_tensor("wo", [DHC, d], F16, kind="ExternalInput")
    out_d = nc.dram_tensor("out", [b, s, d], F16, kind="ExternalOutput")

    with TileContext(nc) as tc:
        with (
            tc.tile_pool(name="const", bufs=1) as cpool,
            tc.tile_pool(name="wpool", bufs=3 * n_kt) as wpool,
            tc.tile_pool(name="xt", bufs=2 * n_kt) as xt_pool,
            tc.tile_pool(name="qk", bufs=2) as qk_pool,
            tc.tile_pool(name="vt", bufs=3) as vt_pool,
            tc.tile_pool(name="at", bufs=2) as at_pool,
            tc.tile_pool(name="pexp", bufs=3) as pexp_pool,
            tc.tile_pool(name="rline", bufs=5) as rline_pool,
            tc.tile_pool(name="small", bufs=10) as small,
            tc.tile_pool(name="osb", bufs=4) as osb_pool,
            tc.tile_pool(name="ps", bufs=1, space="PSUM") as pp,
        ):
            # ---- constants & weights ----
            ident = cpool.tile([128, 128], F32, tag="ident")
            make_identity(nc, ident[:, :])
            ones_col = cpool.tile([128, 32], F16, tag="ones_col")
            nc.vector.memset(ones_col[:, :], 1.0)

            bq = cpool.tile([DHC, 1], F32, tag="bq")
            bk = cpool.tile([DHC, 1], F32, tag="bk")
            nc.sync.dma_start(out=bq[:, :], in_=bq_d[:, :])
            nc.sync.dma_start(out=bk[:, :], in_=bk_d[:, :])

            w16 = {}
            for name, dram in (("q", wq_d), ("k", wk_d), ("v", wv_d)):
                for kt in range(n_kt):
                    wt = wpool.tile([128, DHC], F16, tag="w",
                                    name=f"w_{name}{kt}")
                    nc.sync.dma_start(
                        out=wt[:, :], in_=dram[kt * 128:(kt + 1) * 128, :]
                    )
                    w16[(name, kt)] = wt
            wo = cpool.tile([DHC, d], F16, tag="wo")
            nc.sync.dma_start(out=wo[:, :], in_=wo_d[:, :])

            # stage-D queue: (bi_out, attnT, (rrec_h0, rrec_h1), jc, tt, half)
            d_queue = []

            def emit_d_unit(use_act=False):
                if not d_queue:
                    return
                bi_out, attnT_p, rr, jc_p, tt, half = d_queue.pop(0)
                tl = tt - jc_p * ntj     # column in the per-(jc) rrec tile
                cs = slice(half * 512, (half + 1) * 512)
                poA = pp.tile([128, 512], F32, tag="poA", name="poA")
                poB = pp.tile([128, 512], F32, tag="poB", name="poB")
                nc.tensor.matmul(
                    poA[:, :], attnT_p[0:64, tt * 128:(tt + 1) * 128],
                    wo[0:64, cs], start=True, stop=True,
                    tile_position=(0, 0),
                )
                nc.tensor.matmul(
                    poB[:, :], attnT_p[64:128, tt * 128:(tt + 1) * 128],
                    wo[64:128, cs], start=True, stop=True,
                    tile_position=(64, 0),
                )
                t1 = osb_pool.tile([128, 512], F32, tag="t1", name="t1")
                if use_act:
                    nc.scalar.mul(t1[:, :], poA[:, :], rr[0][:, tl:tl + 1])
                else:
                    nc.vector.tensor_scalar_mul(
                        t1[:, :], poA[:, :], rr[0][:, tl:tl + 1]
                    )
                osb = osb_pool.tile([128, 512], F16, tag="osb", name="osb")
                nc.vector.scalar_tensor_tensor(
                    osb[:, :], poB[:, :], rr[1][:, tl:tl + 1], t1[:, :],
                    ALU.mult, ALU.add,
                )
                nc.sync.dma_start(
                    out=out_d[bi_out, tt * 128:(tt + 1) * 128, cs],
                    in_=osb[:, :],
                )

            # r-path state: rline tiles per (jc, h); rrec per (jc, h)
            def emit_rpath_dma(rline_t):
                """rline [1, qw] -> r16 [ntj, 128] via SBUF->SBUF DMA on the
                ACT hwdge queue (separate from bulk sync-queue traffic)."""
                r16 = small.tile([ntj, 128], F32, tag="r16")
                nc.scalar.dma_start(
                    out=r16[:, :],
                    in_=rline_t[0:1, :].rearrange("a (t c) -> a t c", t=ntj),
                )
                return r16

            def emit_rpath_finish(r16, rrec_t):
                prt = pp.tile([128, ntj], F32, tag="st", bufs=2, name="prt")
                nc.tensor.transpose(
                    prt[:, :], r16[:, :], ident[0:ntj, 0:ntj]
                )
                nc.vector.reciprocal(rrec_t[:, :], prt[:, :])

            pend_r = {}       # (jc, h) -> (r16, rrec) awaiting transpose
            prev_tail = None  # (attnT, rrec-pair, jc) of prev batch's last jc

            for bi in range(b):
                # ---- stage A: x^T DMA in 512-col chunks, kt-major ----
                xT = [xt_pool.tile([128, s], F16, tag="xt", name=f"xT{kt}")
                      for kt in range(n_kt)]

                def emit_x_chunk(c4):
                    lo, hi = c4 * 512, (c4 + 1) * 512
                    for kt in range(n_kt):
                        nc.sync.dma_start(
                            out=xT[kt][:, lo:hi],
                            in_=x_d[bi, kt * 128:(kt + 1) * 128, lo:hi],
                        )

                # ---- stage B: Q^T, K^T projections -> fp8 [64, 2, s] ----
                qT8 = qk_pool.tile([64, 2 * s], F8, tag="qT8")
                kT8 = qk_pool.tile([64, 2 * s], F8, tag="kT8")
                q8v = qT8.rearrange("p (two s) -> p two s", two=2)
                k8v = kT8.rearrange("p (two s) -> p two s", two=2)
                n_c = s // qw
                for c in range(n_c):
                    emit_x_chunk(2 * c)
                    emit_x_chunk(2 * c + 1)
                    for name, dstv, bias in (("q", q8v, bq), ("k", k8v, bk)):
                        ppr = pp.tile([128, qw], F32, tag="st", bufs=2,
                                      name="ppr")
                        for kt in range(n_kt):
                            for j in range(qw // 512):
                                nc.tensor.matmul(
                                    ppr[:, j * 512:(j + 1) * 512],
                                    w16[(name, kt)][:, :],
                                    xT[kt][:, c * qw + j * 512:
                                            c * qw + (j + 1) * 512],
                                    start=(kt == 0),
                                    stop=(kt == n_kt - 1),
                                )
                        for i in range(2):
                            nc.vector.tensor_scalar_add(
                                dstv[0:64, i, c * qw:(c + 1) * qw],
                                ppr[64 * i:64 * (i + 1), :],
                                bias[64 * i:64 * (i + 1), 0:1],
                            )

                # V natural, interleaved-head layout [V_A |1| V_B |1] / 130
                vt = vt_pool.tile([128, n_tt * 130], F16, tag="vt")
                ones_dst = vt.rearrange("p (t two sv) -> p t two sv",
                                        two=2, sv=65)[:, :, :, 64]
                nc.vector.tensor_copy(ones_dst, ones_col[:, 0:2 * n_tt]
                                      .rearrange("p (t two) -> p t two", two=2))
                for tt in range(n_tt):
                    pv = pp.tile([128, 128], F32, tag="poA", name="pv")
                    for kt in range(n_kt):
                        nc.tensor.matmul(
                            pv[:, :],
                            xT[kt][:, tt * 128:(tt + 1) * 128],
                            w16[("v", kt)][:, :],
                            start=(kt == 0),
                            stop=(kt == n_kt - 1),
                        )
                    vdst = vt.rearrange("p (t two sv) -> p t two sv",
                                        two=2, sv=65)[:, tt, :, 0:64]
                    nc.vector.tensor_copy(
                        vdst, pv.rearrange("p (two sv) -> p two sv", two=2)
                    )

                # finish the r-path of the previous batch's last jc (the DMA
                # has had all of stage B to land) and queue its D units
                if prev_tail is not None:
                    bi_p, attnT_p, jc_p = prev_tail
                    rrs = []
                    for h in range(HPC):
                        r16, rrec_t = pend_r.pop((jc_p, h))
                        emit_rpath_finish(r16, rrec_t)
                        rrs.append(rrec_t)
                    for tt in range(jc_p * ntj, (jc_p + 1) * ntj):
                        for half in range(2):
                            d_queue.append(
                                (bi_p, attnT_p, (rrs[0], rrs[1]),
                                 jc_p, tt, half))
                    prev_tail = None

                # ---- stage C: attention, jc outer / h inner ----
                attnT = at_pool.tile([DHC, s], F16, tag="attnT")
                vtv = vt.rearrange("p (t two sv) -> p t two sv", two=2, sv=65)
                for jc in range(n_jc):
                    for h in range(HPC):
                        hs = slice(h * 64, (h + 1) * 64)
                        hp = slice(h * 32, (h + 1) * 32)
                        slot_base = jc * HPC * (n_tt + 1) + h * (n_tt + 1)
                        rline = rline_pool.tile([1, qw], F32, tag="rline")
                        qs = slice(jc * qw, (jc + 1) * qw)
                        av = pp.tile([65, qw], F32, tag="av", name="av")
                        pexps = {}
                        for kt in range(n_tt + 1):
                            if kt < n_tt:
                                st = pp.tile([128, qw], F32, tag="st",
                                             bufs=2, name="st")
                                for j in range(qw // 512):
                                    nc.tensor.matmul(
                                        st[:, j * 512:(j + 1) * 512],
                                        k8v[hp, :, kt * 128:(kt + 1) * 128],
                                        q8v[hp, :, jc * qw + j * 512:
                                            jc * qw + (j + 1) * 512],
                                        start=True, stop=True,
                                        perf_mode=DR,
                                    )
                                # deferred r-transposes for this batch's jc0
                                if jc == 1 and h == 0 and kt == 2:
                                    rrs = []
                                    for hh in range(HPC):
                                        r16, rrec_t = pend_r.pop((0, hh))
                                        emit_rpath_finish(r16, rrec_t)
                                        rrs.append(rrec_t)
                                    for tt in range(ntj):
                                        for half in range(2):
                                            d_queue.append(
                                                (bi, attnT, (rrs[0], rrs[1]),
                                                 0, tt, half))
                                # pop D units: every other slot early, every
                                # slot in the last (jc, h) chunk
                                if (jc == n_jc - 1 and h == HPC - 1) \
                                        or (slot_base + kt) % 2 == 0:
                                    emit_d_unit()
                                pexp = pexp_pool.tile([128, qw], F16,
                                                      tag="pexp", name="pexp")
                                nc.scalar.activation(
                                    pexp[:, :], st[:, :], AF.Exp,
                                    scale=0.125 / (QK_SCALE * QK_SCALE),
                                )
                                pexps[kt] = pexp
                            if kt > 0:
                                px = pexps.pop(kt - 1)
                                for j in range(qw // 512):
                                    nc.tensor.matmul(
                                        av[:, j * 512:(j + 1) * 512],
                                        vtv[:, kt - 1, h, :],
                                        px[:, j * 512:(j + 1) * 512],
                                        start=(kt == 1),
                                        stop=(kt == n_tt),
                                    )
                        nc.vector.tensor_copy(attnT[hs, qs], av[0:64, :])
                        nc.vector.tensor_copy(rline[0:1, :], av[64:65, :])
                        rrec_t = small.tile([128, ntj], F32, tag="rrec",
                                            name=f"rrec{jc}{h}")
                        pend_r[(jc, h)] = (emit_rpath_dma(rline), rrec_t)

                prev_tail = (bi, attnT, n_jc - 1)

            # ---- tail: finish last batch's final jc and flush its D ----
            bi_p, attnT_p, jc_p = prev_tail
            rrs = []
            for h in range(HPC):
                r16, rrec_t = pend_r.pop((jc_p, h))
                emit_rpath_finish(r16, rrec_t)
                rrs.append(rrec_t)
            for tt in range(jc_p * ntj, (jc_p + 1) * ntj):
                for half in range(2):
                    d_queue.append(
                        (bi_p, attnT_p, (rrs[0], rrs[1]), jc_p, tt, half))
            while d_queue:
                emit_d_unit(use_act=True)

    nc.compile()
    return nc


def _get_nc(b, s):
    key = (b, s)
    if key not in _CACHE:
        _CACHE[key] = build_nc(b, s)
    return _CACHE[key]


# PSUM partition order for the QK projections: [h0 d0-31 | h1 d0-31 |
# h0 d32-63 | h1 d32-63] so two partition-shift DVE ops produce the
# [32, 2, S]-per-head fp8 layout DoubleRow needs.
_PERM = np.concatenate([np.arange(0, 32), np.arange(64, 96),
                        np.arange(32, 64), np.arange(96, 128)])


def make_in_maps(x, w_q, b_q, w_k, b_k, w_v, w_o):
    x16 = np.ascontiguousarray(
        np.asarray(x, dtype=np.float16).transpose(0, 2, 1))
    wq32 = np.asarray(w_q, dtype=np.float32) * QK_SCALE
    wk32 = np.asarray(w_k, dtype=np.float32) * QK_SCALE
    bq32 = np.asarray(b_q, dtype=np.float32) * QK_SCALE
    bk32 = np.asarray(b_k, dtype=np.float32) * QK_SCALE
    wv16 = np.asarray(w_v, dtype=np.float16)
    wo16 = np.asarray(w_o, dtype=np.float16)
    in_maps = []
    for i in range(NCORES):
        cs = slice(i * DHC, (i + 1) * DHC)
        wq_c = wq32[:, cs][:, _PERM].astype(np.float16)
        wk_c = wk32[:, cs][:, _PERM].astype(np.float16)
        bq_c = bq32[cs][_PERM, None]
        bk_c = bk32[cs][_PERM, None]
        in_maps.append({
            "x": x16,
            "wq": np.ascontiguousarray(wq_c),
            "wk": np.ascontiguousarray(wk_c),
            "wv": np.ascontiguousarray(wv16[:, cs]),
            "bq": np.ascontiguousarray(bq_c, dtype=np.float32),
            "bk": np.ascontiguousarray(bk_c, dtype=np.float32),
            "wo": np.ascontiguousarray(wo16[cs, :]),
        })
    return in_maps


def kernel(x, w_q, b_q, w_k, b_k, w_v, b_v, w_o, b_o, _trace=False):
    x = np.asarray(x, dtype=np.float32)
    nc = _get_nc(x.shape[0], x.shape[1])
    in_maps = make_in_maps(x, w_q, b_q, w_k, b_k, w_v, w_o)
    kw = {}
    if _trace:
        import tempfile
        kw = dict(trace=True, trace_cores=list(range(NCORES)),
                  tmpdir=tempfile.mkdtemp(prefix="mha_trace_"))
    res = bass_utils.run_bass_kernel_spmd(
        nc, in_maps, core_ids=list(range(NCORES)), **kw
    )
    out = np.zeros(x.shape, dtype=np.float32)
    for i in range(NCORES):
        out += np.asarray(res.results[i]["out"], dtype=np.float32)
    out += np.asarray(b_o, dtype=np.float32)[None, None, :]
    out += (np.asarray(b_v, dtype=np.float32)
            @ np.asarray(w_o, dtype=np.float32))[None, None, :]
    if _trace:
        return out, res
    return out


# revision 15
# speedup vs baseline: 1.2588x; 1.2588x over previous
"""Multi-head attention on 8 Trainium2 NeuronCores — fp16 pipeline.

Sharding: tensor-parallel over heads (2 heads/core), full batch on every
core; host sums the 8 partial outputs and adds b_o + b_v @ w_o.

All matmul operands fp16 (1 cycle/row; fp8 DoubleRow was measured to
give no gain at contraction=64 — the PE is output-row bound, not
contraction bound — while doubling LDWEIGHTS serialization).

Pipeline (per batch bi):
  A: xT[kt] [128, S] fp16 DMA'd in 1024-col chunks (2KB/partition
     lines keep DMA at full rate), kt-major per chunk, so batch-0
     projections start after 2MB instead of 4MB.
  B: QK projections (fp16 MM -> PSUM; DVE bias-add -> fp16),
     V natural via lhsT=xT tile.
  C: jc-OUTER (2 chunks of 1024 q), h inner; per kt: S^T MM,
     D-unit pop (rate-controlled: every other slot, every slot in the
     final (jc,h) chunk), exp (ACT), AV fp16 MM with ones row -> r.
     rline r-path: SBUF->SBUF DMA on the ACT hwdge queue (no DRAM
     roundtrip, no head-of-line behind bulk x/out DMAs); PE transpose +
     reciprocal deferred to where the DMA has had time to land.
  D: per (tt, half): poA/poB = attnT_h^T @ w_o_h (tile_position rows),
     t1 = poA*rrec0 (DVE; ACT for the tail flush), osb = poB*rrec1 + t1
     -> fp16 -> DMA out. D(bi)-jc0 units interleave into C(bi)-jc1;
     D(bi)-jc1 units into C(bi+1)-jc0; only b3-jc1 flushes at the end.
"""

import numpy as np

import concourse.bacc as bacc
import concourse.mybir as mybir
from concourse.tile import TileContext
from concourse.masks import make_identity
from concourse import bass_utils

dt = mybir.dt
F32 = dt.float32
F16 = dt.float16
AF = mybir.ActivationFunctionType
ALU = mybir.AluOpType

B, S, D = 4, 2048, 1024
H, DH = 16, 64
NCORES = 8
HPC = H // NCORES          # heads per core = 2
DHC = HPC * DH             # 128 projection cols per core

_CACHE = {}


def build_nc(b=B, s=S):
    d = D
    n_tt = s // 128            # token tiles per batch
    n_kt = d // 128            # contraction tiles for projections
    qw = 1024 if s >= 1024 else s
    n_jc = s // qw             # q chunks per batch
    ntj = qw // 128            # token tiles per q chunk
    assert s % 512 == 0 and d == 1024
    assert n_jc == 2, "C-stage interleave schedule assumes two q chunks"

    nc = bacc.Bacc("TRN2", target_bir_lowering=False, debug=False)

    x_d = nc.dram_tensor("x", [b, d, s], F16, kind="ExternalInput")
    wq_d = nc.dram_tensor("wq", [d, DHC], F16, kind="ExternalInput")
    wk_d = nc.dram_tensor("wk", [d, DHC], F16, kind="ExternalInput")
    wv_d = nc.dram_tensor("wv", [d, DHC], F16, kind="ExternalInput")
    bq_d = nc.dram_tensor("bq", [DHC, 1], F32, kind="ExternalInput")
    bk_d = nc.dram_tensor("bk", [DHC, 1], F32, kind="ExternalInput")
    wo_d = nc.dram_tensor("wo", [DHC, d], F16, kind="ExternalInput")
    out_d = nc.dram_tensor("out", [b, s, d], F16, kind="ExternalOutput")

    with TileContext(nc) as tc:
        with (
            tc.tile_pool(name="const", bufs=1) as cpool,
            tc.tile_pool(name="wpool", bufs=3 * n_kt) as wpool,
            tc.tile_pool(name="xt", bufs=2 * n_kt) as xt_pool,
            tc.tile_pool(name="qk", bufs=2) as qk_pool,
            tc.tile_pool(name="vt", bufs=3) as vt_pool,
            tc.tile_pool(name="at", bufs=2) as at_pool,
            tc.tile_pool(name="pexp", bufs=3) as pexp_pool,
            tc.tile_pool(name="rline", bufs=5) as rline_pool,
            tc.tile_pool(name="small", bufs=10) as small,
            tc.tile_pool(name="osb", bufs=4) as osb_pool,
            tc.tile_pool(name="ps", bufs=1, space="PSUM") as pp,
        ):
            # ---- constants & weights ----
            ident = cpool.tile([128, 128], F32, tag="ident")
            make_identity(nc, ident[:, :])
            ones_col = cpool.tile([128, 32], F16, tag="ones_col")
            nc.vector.memset(ones_col[:, :], 1.0)

            bq = cpool.tile([DHC, 1], F32, tag="bq")
            bk = cpool.tile([DHC, 1], F32, tag="bk")
            nc.sync.dma_start(out=bq[:, :], in_=bq_d[:, :])
            nc.sync.dma_start(out=bk[:, :], in_=bk_d[:, :])

            w16 = {}
            for name, dram in (("q", wq_d), ("k", wk_d), ("v", wv_d)):
                for kt in range(n_kt):
                    wt = wpool.tile([128, DHC], F16, tag="w",
                                    name=f"w_{name}{kt}")
                    nc.sync.dma_start(
                        out=wt[:, :], in_=dram[kt * 128:(kt + 1) * 128, :]
                    )
                    w16[(name, kt)] = wt
            wo = cpool.tile([DHC, d], F16, tag="wo")
            nc.sync.dma_start(out=wo[:, :], in_=wo_d[:, :])

            # stage-D queue: (bi_out, attnT, (rrec_h0, rrec_h1), jc, tt, half)
            d_queue = []

            def emit_d_unit(use_act=False):
                if not d_queue:
                    return
                bi_out, attnT_p, rr, jc_p, tt, half = d_queue.pop(0)
                tl = tt - jc_p * ntj     # column in the per-(jc) rrec tile
                cs = slice(half * 512, (half + 1) * 512)
                poA = pp.tile([128, 512], F32, tag="poA", name="poA")
                poB = pp.tile([128, 512], F32, tag="poB", name="poB")
                nc.tensor.matmul(
                    poA[:, :], attnT_p[0:64, tt * 128:(tt + 1) * 128],
                    wo[0:64, cs], start=True, stop=True,
                    tile_position=(0, 0),
                )
                nc.tensor.matmul(
                    poB[:, :], attnT_p[64:128, tt * 128:(tt + 1) * 128],
                    wo[64:128, cs], start=True, stop=True,
                    tile_position=(64, 0),
                )
                t1 = osb_pool.tile([128, 512], F32, tag="t1", name="t1")
                if use_act:
                    nc.scalar.mul(t1[:, :], poA[:, :], rr[0][:, tl:tl + 1])
                else:
                    nc.vector.tensor_scalar_mul(
                        t1[:, :], poA[:, :], rr[0][:, tl:tl + 1]
                    )
                osb = osb_pool.tile([128, 512], F16, tag="osb", name="osb")
                nc.vector.scalar_tensor_tensor(
                    osb[:, :], poB[:, :], rr[1][:, tl:tl + 1], t1[:, :],
                    ALU.mult, ALU.add,
                )
                nc.sync.dma_start(
                    out=out_d[bi_out, tt * 128:(tt + 1) * 128, cs],
                    in_=osb[:, :],
                )

            # r-path state: rline tiles per (jc, h); rrec per (jc, h)
            def emit_rpath_dma(rline_t):
                """rline [1, qw] -> r16 [ntj, 128] via SBUF->SBUF DMA on the
                ACT hwdge queue (separate from bulk sync-queue traffic)."""
                r16 = small.tile([ntj, 128], F32, tag="r16")
                nc.scalar.dma_start(
                    out=r16[:, :],
                    in_=rline_t[0:1, :].rearrange("a (t c) -> a t c", t=ntj),
                )
                return r16

            def emit_rpath_finish(r16, rrec_t):
                prt = pp.tile([128, ntj], F32, tag="st", bufs=2, name="prt")
                nc.tensor.transpose(
                    prt[:, :], r16[:, :], ident[0:ntj, 0:ntj]
                )
                nc.vector.reciprocal(rrec_t[:, :], prt[:, :])

            pend_r = {}       # (jc, h) -> (r16, rrec) awaiting transpose
            prev_tail = None  # (attnT, rrec-pair, jc) of prev batch's last jc

            for bi in range(b):
                # ---- stage A: x^T DMA in 1024-col chunks (2KB/partition
                # lines keep the DMA at full rate), kt-major per chunk ----
                xT = [xt_pool.tile([128, s], F16, tag="xt", name=f"xT{kt}")
                      for kt in range(n_kt)]

                def emit_x_chunk(c4):
                    lo, hi = c4 * qw, (c4 + 1) * qw
                    for kt in range(n_kt):
                        nc.sync.dma_start(
                            out=xT[kt][:, lo:hi],
                            in_=x_d[bi, kt * 128:(kt + 1) * 128, lo:hi],
                        )

                # ---- stage B: Q^T, K^T projections (fp16) ----
                qT = qk_pool.tile([DHC, s], F16, tag="qT")
                kT = qk_pool.tile([DHC, s], F16, tag="kT")
                n_c = s // qw
                for c in range(n_c):
                    emit_x_chunk(c)
                    for name, dst, bias in (("q", qT, bq), ("k", kT, bk)):
                        ppr = pp.tile([128, qw], F32, tag="st", bufs=2,
                                      name="ppr")
                        for kt in range(n_kt):
                            for j in range(qw // 512):
                                nc.tensor.matmul(
                                    ppr[:, j * 512:(j + 1) * 512],
                                    w16[(name, kt)][:, :],
                                    xT[kt][:, c * qw + j * 512:
                                            c * qw + (j + 1) * 512],
                                    start=(kt == 0),
                                    stop=(kt == n_kt - 1),
                                )
                        nc.vector.tensor_scalar_add(
                            dst[:, c * qw:(c + 1) * qw], ppr[:, :],
                            bias[:, 0:1],
                        )

                # V natural, interleaved-head layout [V_A |1| V_B |1] / 130
                vt = vt_pool.tile([128, n_tt * 130], F16, tag="vt")
                ones_dst = vt.rearrange("p (t two sv) -> p t two sv",
                                        two=2, sv=65)[:, :, :, 64]
                nc.vector.tensor_copy(ones_dst, ones_col[:, 0:2 * n_tt]
                                      .rearrange("p (t two) -> p t two", two=2))
                for tt in range(n_tt):
                    pv = pp.tile([128, 128], F32, tag="poA", name="pv")
                    for kt in range(n_kt):
                        nc.tensor.matmul(
                            pv[:, :],
                            xT[kt][:, tt * 128:(tt + 1) * 128],
                            w16[("v", kt)][:, :],
                            start=(kt == 0),
                            stop=(kt == n_kt - 1),
                        )
                    vdst = vt.rearrange("p (t two sv) -> p t two sv",
                                        two=2, sv=65)[:, tt, :, 0:64]
                    nc.vector.tensor_copy(
                        vdst, pv.rearrange("p (two sv) -> p two sv", two=2)
                    )

                # finish the r-path of the previous batch's last jc (the DMA
                # has had all of stage B to land) and queue its D units
                if prev_tail is not None:
                    bi_p, attnT_p, jc_p = prev_tail
                    rrs = []
                    for h in range(HPC):
                        r16, rrec_t = pend_r.pop((jc_p, h))
                        emit_rpath_finish(r16, rrec_t)
                        rrs.append(rrec_t)
                    for tt in range(jc_p * ntj, (jc_p + 1) * ntj):
                        for half in range(2):
                            d_queue.append(
                                (bi_p, attnT_p, (rrs[0], rrs[1]),
                                 jc_p, tt, half))
                    prev_tail = None

                # ---- stage C: attention, jc outer / h inner ----
                attnT = at_pool.tile([DHC, s], F16, tag="attnT")
                vtv = vt.rearrange("p (t two sv) -> p t two sv", two=2, sv=65)
                for jc in range(n_jc):
                    for h in range(HPC):
                        hs = slice(h * 64, (h + 1) * 64)
                        slot_base = jc * HPC * (n_tt + 1) + h * (n_tt + 1)
                        rline = rline_pool.tile([1, qw], F32, tag="rline")
                        qs = slice(jc * qw, (jc + 1) * qw)
                        av = pp.tile([65, qw], F32, tag="av", name="av")
                        pexps = {}
                        for kt in range(n_tt + 1):
                            if kt < n_tt:
                                st = pp.tile([128, qw], F32, tag="st",
                                             bufs=2, name="st")
                                for j in range(qw // 512):
                                    nc.tensor.matmul(
                                        st[:, j * 512:(j + 1) * 512],
                                        kT[hs, kt * 128:(kt + 1) * 128],
                                        qT[hs, jc * qw + j * 512:
                                           jc * qw + (j + 1) * 512],
                                        start=True, stop=True,
                                    )
                                # deferred r-transposes for this batch's jc0
                                if jc == 1 and h == 0 and kt == 2:
                                    rrs = []
                                    for hh in range(HPC):
                                        r16, rrec_t = pend_r.pop((0, hh))
                                        emit_rpath_finish(r16, rrec_t)
                                        rrs.append(rrec_t)
                                    for tt in range(ntj):
                                        for half in range(2):
                                            d_queue.append(
                                                (bi, attnT, (rrs[0], rrs[1]),
                                                 0, tt, half))
                                # pop D units: every other slot early, every
                                # slot in the last (jc, h) chunk
                                if (jc == n_jc - 1 and h == HPC - 1) \
                                        or (slot_base + kt) % 2 == 0:
                                    emit_d_unit()
                                pexp = pexp_pool.tile([128, qw], F16,
                                                      tag="pexp", name="pexp")
                                nc.scalar.activation(
                                    pexp[:, :], st[:, :], AF.Exp, scale=0.125
                                )
                                pexps[kt] = pexp
                            if kt > 0:
                                px = pexps.pop(kt - 1)
                                for j in range(qw // 512):
                                    nc.tensor.matmul(
                                        av[:, j * 512:(j + 1) * 512],
                                        vtv[:, kt - 1, h, :],
                                        px[:, j * 512:(j + 1) * 512],
                                        start=(kt == 1),
                                        stop=(kt == n_tt),
                                    )
                        nc.vector.tensor_copy(attnT[hs, qs], av[0:64, :])
                        nc.vector.tensor_copy(rline[0:1, :], av[64:65, :])
                        rrec_t = small.tile([128, ntj], F32, tag="rrec",
                                            name=f"rrec{jc}{h}")
                        pend_r[(jc, h)] = (emit_rpath_dma(rline), rrec_t)

                prev_tail = (bi, attnT, n_jc - 1)

            # ---- tail: finish last batch's final jc and flush its D ----
            bi_p, attnT_p, jc_p = prev_tail
            rrs = []
            for h in range(HPC):
                r16, rrec_t = pend_r.pop((jc_p, h))
                emit_rpath_finish(r16, rrec_t)
                rrs.append(rrec_t)
            for tt in range(jc_p * ntj, (jc_p + 1) * ntj):
                for half in range(2):
                    d_queue.append(
                        (bi_p, attnT_p, (rrs[0], rrs[1]), jc_p, tt, half))
            while d_queue:
                emit_d_unit(use_act=True)

    nc.compile()
    return nc


def _get_nc(b, s):
    key = (b, s)
    if key not in _CACHE:
        _CACHE[key] = build_nc(b, s)
    return _CACHE[key]


def make_in_maps(x, w_q, b_q, w_k, b_k, w_v, w_o):
    x16 = np.ascontiguousarray(
        np.asarray(x, dtype=np.float16).transpose(0, 2, 1))
    wq16 = np.asarray(w_q, dtype=np.float16)
    wk16 = np.asarray(w_k, dtype=np.float16)
    wv16 = np.asarray(w_v, dtype=np.float16)
    wo16 = np.asarray(w_o, dtype=np.float16)
    in_maps = []
    for i in range(NCORES):
        cs = slice(i * DHC, (i + 1) * DHC)
        in_maps.append({
            "x": x16,
            "wq": np.ascontiguousarray(wq16[:, cs]),
            "wk": np.ascontiguousarray(wk16[:, cs]),
            "wv": np.ascontiguousarray(wv16[:, cs]),
            "bq": np.ascontiguousarray(b_q[cs, None], dtype=np.float32),
            "bk": np.ascontiguousarray(b_k[cs, None], dtype=np.float32),
            "wo": np.ascontiguousarray(wo16[cs, :]),
        })
    return in_maps


def kernel(x, w_q, b_q, w_k, b_k, w_v, b_v, w_o, b_o, _trace=False):
    x = np.asarray(x, dtype=np.float32)
    nc = _get_nc(x.shape[0], x.shape[1])
    in_maps = make_in_maps(x, w_q, b_q, w_k, b_k, w_v, w_o)
    kw = {}
    if _trace:
        import tempfile
        kw = dict(trace=True, trace_cores=list(range(NCORES)),
                  tmpdir=tempfile.mkdtemp(prefix="mha_trace_"))
    res = bass_utils.run_bass_kernel_spmd(
        nc, in_maps, core_ids=list(range(NCORES)), **kw
    )
    out = np.zeros(x.shape, dtype=np.float32)
    for i in range(NCORES):
        out += np.asarray(res.results[i]["out"], dtype=np.float32)
    out += np.asarray(b_o, dtype=np.float32)[None, None, :]
    out += (np.asarray(b_v, dtype=np.float32)
            @ np.asarray(w_o, dtype=np.float32))[None, None, :]
    if _trace:
        return out, res
    return out


# revision 21
# speedup vs baseline: 1.2907x; 1.0253x over previous
"""Multi-head attention on 8 Trainium2 NeuronCores — fp16 pipeline.

Sharding: tensor-parallel over heads (2 heads/core), full batch on every
core; host sums the 8 partial outputs and adds b_o + b_v @ w_o.

All matmul operands fp16 (1 cycle/row; fp8 DoubleRow was measured to
give no gain at contraction=64 — the PE is output-row bound, not
contraction bound — while doubling LDWEIGHTS serialization).

Pipeline (per batch bi):
  A: xT[kt] [128, S] fp16 DMA'd in 1024-col chunks (2KB/partition
     lines keep DMA at full rate), kt-major per chunk, so batch-0
     projections start after 2MB instead of 4MB.
  B: QK projections (fp16 MM -> PSUM; DVE bias-add -> fp16),
     V natural via lhsT=xT tile.
  C: jc-OUTER (2 chunks of 1024 q), h inner; per kt: S^T MM,
     D-unit pop (rate-controlled: every other slot, every slot in the
     final (jc,h) chunk), exp (ACT), AV fp16 MM with ones row -> r.
     rline r-path: SBUF->SBUF DMA on the ACT hwdge queue (no DRAM
     roundtrip, no head-of-line behind bulk x/out DMAs); PE transpose +
     reciprocal deferred to where the DMA has had time to land.
  D: per (tt, half): poA/poB = attnT_h^T @ w_o_h (tile_position rows),
     t1 = poA*rrec0 (DVE; ACT for the tail flush), osb = poB*rrec1 + t1
     -> fp16 -> DMA out. D(bi)-jc0 units interleave into C(bi)-jc1;
     D(bi)-jc1 units into C(bi+1)-jc0; only b3-jc1 flushes at the end.
"""

import numpy as np

import concourse.bacc as bacc
import concourse.mybir as mybir
from concourse.tile import TileContext
from concourse import bass_utils

dt = mybir.dt
F32 = dt.float32
F16 = dt.float16
AF = mybir.ActivationFunctionType
ALU = mybir.AluOpType

B, S, D = 4, 2048, 1024
H, DH = 16, 64
NCORES = 8
HPC = H // NCORES          # heads per core = 2
DHC = HPC * DH             # 128 projection cols per core

_CACHE = {}


def build_nc(b=B, s=S):
    d = D
    n_tt = s // 128            # token tiles per batch
    n_kt = d // 128            # contraction tiles for projections
    qw = 1024 if s >= 1024 else s
    n_jc = s // qw             # q chunks per batch
    ntj = qw // 128            # token tiles per q chunk
    assert s % 512 == 0 and d == 1024
    assert n_jc == 2, "C-stage interleave schedule assumes two q chunks"

    nc = bacc.Bacc("TRN2", target_bir_lowering=False, debug=False)

    x_d = nc.dram_tensor("x", [b, d, s], F16, kind="ExternalInput")
    wq_d = nc.dram_tensor("wq", [d, DHC], F16, kind="ExternalInput")
    wk_d = nc.dram_tensor("wk", [d, DHC], F16, kind="ExternalInput")
    wv_d = nc.dram_tensor("wv", [d, DHC], F16, kind="ExternalInput")
    bq_d = nc.dram_tensor("bq", [DHC, 1], F32, kind="ExternalInput")
    bk_d = nc.dram_tensor("bk", [DHC, 1], F32, kind="ExternalInput")
    wo_d = nc.dram_tensor("wo", [DHC, d], F16, kind="ExternalInput")
    out_d = nc.dram_tensor("out", [b, s, d], F16, kind="ExternalOutput")

    with TileContext(nc) as tc:
        with (
            tc.tile_pool(name="const", bufs=1) as cpool,
            tc.tile_pool(name="wpool", bufs=3 * n_kt) as wpool,
            tc.tile_pool(name="xt", bufs=2 * n_kt) as xt_pool,
            tc.tile_pool(name="qk", bufs=2) as qk_pool,
            tc.tile_pool(name="vt", bufs=3) as vt_pool,
            tc.tile_pool(name="at", bufs=2) as at_pool,
            tc.tile_pool(name="pexp", bufs=3) as pexp_pool,
            tc.tile_pool(name="rline", bufs=5) as rline_pool,
            tc.tile_pool(name="osb", bufs=4) as osb_pool,
            tc.tile_pool(name="ps", bufs=1, space="PSUM") as pp,
        ):
            # ---- constants & weights ----
            ones_col = cpool.tile([128, 32], F16, tag="ones_col")
            nc.vector.memset(ones_col[:, :], 1.0)

            bq = cpool.tile([DHC, 1], F32, tag="bq")
            bk = cpool.tile([DHC, 1], F32, tag="bk")
            nc.sync.dma_start(out=bq[:, :], in_=bq_d[:, :])
            nc.sync.dma_start(out=bk[:, :], in_=bk_d[:, :])

            w16 = {}
            for name, dram in (("q", wq_d), ("k", wk_d), ("v", wv_d)):
                for kt in range(n_kt):
                    wt = wpool.tile([128, DHC], F16, tag="w",
                                    name=f"w_{name}{kt}")
                    nc.sync.dma_start(
                        out=wt[:, :], in_=dram[kt * 128:(kt + 1) * 128, :]
                    )
                    w16[(name, kt)] = wt
            wo = cpool.tile([DHC, d], F16, tag="wo")
            nc.sync.dma_start(out=wo[:, :], in_=wo_d[:, :])

            # stage-D queue: (bi_out, attnT, tt, half); attnT is normalized
            d_queue = []

            def emit_d_unit(use_act=False):
                if not d_queue:
                    return
                bi_out, attnT_p, tt, half = d_queue.pop(0)
                cs = slice(half * 512, (half + 1) * 512)
                po = pp.tile([128, 512], F32, tag="poA", bufs=2, name="po")
                nc.tensor.matmul(
                    po[:, :], attnT_p[:, tt * 128:(tt + 1) * 128],
                    wo[:, cs], start=True, stop=True,
                )
                osb = osb_pool.tile([128, 512], F16, tag="osb", name="osb")
                if use_act:
                    nc.scalar.copy(osb[:, :], po[:, :])
                else:
                    nc.vector.tensor_copy(osb[:, :], po[:, :])
                nc.sync.dma_start(
                    out=out_d[bi_out, tt * 128:(tt + 1) * 128, cs],
                    in_=osb[:, :],
                )

            def emit_norm(rline_t, attnT_t, jc, h):
                """1/rline broadcast down 64 partitions, then scale the raw
                attnT rows of head h in place."""
                rrow = rline_pool.tile([1, qw], F32, tag="rrow", bufs=3)
                nc.vector.reciprocal(rrow[0:1, :], rline_t[0:1, :])
                rb = rline_pool.tile([128, qw], F32, tag="rb", bufs=3)
                nc.gpsimd.partition_broadcast(rb[:, :], rrow[0:1, :])
                hs = slice(h * 64, (h + 1) * 64)
                qs = slice(jc * qw, (jc + 1) * qw)
                nc.vector.tensor_tensor(
                    attnT_t[hs, qs], attnT_t[hs, qs], rb[hs, :], ALU.mult
                )

            pend_norm = {}    # (jc, h) -> (rline, attnT) awaiting normalize
            prev_tail = None  # (bi, attnT, jc) of prev batch's last jc

            for bi in range(b):
                # ---- stage A: x^T DMA in 1024-col chunks (2KB/partition
                # lines keep the DMA at full rate), kt-major per chunk ----
                xT = [xt_pool.tile([128, s], F16, tag="xt", name=f"xT{kt}")
                      for kt in range(n_kt)]

                def emit_x_chunk(c4):
                    lo, hi = c4 * qw, (c4 + 1) * qw
                    for kt in range(n_kt):
                        nc.sync.dma_start(
                            out=xT[kt][:, lo:hi],
                            in_=x_d[bi, kt * 128:(kt + 1) * 128, lo:hi],
                        )

                # ---- stage B: Q^T, K^T projections (fp16) ----
                qT = qk_pool.tile([DHC, s], F16, tag="qT")
                kT = qk_pool.tile([DHC, s], F16, tag="kT")
                n_c = s // qw
                for c in range(n_c):
                    emit_x_chunk(c)
                    for name, dst, bias in (("q", qT, bq), ("k", kT, bk)):
                        ppr = pp.tile([128, qw], F32, tag="st", bufs=2,
                                      name="ppr")
                        for kt in range(n_kt):
                            for j in range(qw // 512):
                                nc.tensor.matmul(
                                    ppr[:, j * 512:(j + 1) * 512],
                                    w16[(name, kt)][:, :],
                                    xT[kt][:, c * qw + j * 512:
                                            c * qw + (j + 1) * 512],
                                    start=(kt == 0),
                                    stop=(kt == n_kt - 1),
                                )
                        nc.vector.tensor_scalar_add(
                            dst[:, c * qw:(c + 1) * qw], ppr[:, :],
                            bias[:, 0:1],
                        )

                # V natural, interleaved-head layout [V_A |1| V_B |1] / 130
                vt = vt_pool.tile([128, n_tt * 130], F16, tag="vt")
                ones_dst = vt.rearrange("p (t two sv) -> p t two sv",
                                        two=2, sv=65)[:, :, :, 64]
                nc.vector.tensor_copy(ones_dst, ones_col[:, 0:2 * n_tt]
                                      .rearrange("p (t two) -> p t two", two=2))
                for tt in range(n_tt):
                    pv = pp.tile([128, 128], F32, tag="poA", bufs=2, name="pv")
                    for kt in range(n_kt):
                        nc.tensor.matmul(
                            pv[:, :],
                            xT[kt][:, tt * 128:(tt + 1) * 128],
                            w16[("v", kt)][:, :],
                            start=(kt == 0),
                            stop=(kt == n_kt - 1),
                        )
                    vdst = vt.rearrange("p (t two sv) -> p t two sv",
                                        two=2, sv=65)[:, tt, :, 0:64]
                    nc.vector.tensor_copy(
                        vdst, pv.rearrange("p (two sv) -> p two sv", two=2)
                    )

                # normalize the previous batch's last-jc attnT rows and
                # queue its D units
                if prev_tail is not None:
                    bi_p, attnT_p, jc_p = prev_tail
                    for h in range(HPC):
                        rline_t, at_t = pend_norm.pop((jc_p, h))
                        emit_norm(rline_t, at_t, jc_p, h)
                    for tt in range(jc_p * ntj, (jc_p + 1) * ntj):
                        for half in range(2):
                            d_queue.append((bi_p, attnT_p, tt, half))
                    prev_tail = None

                # ---- stage C: attention, jc outer / h inner ----
                attnT = at_pool.tile([DHC, s], F16, tag="attnT")
                vtv = vt.rearrange("p (t two sv) -> p t two sv", two=2, sv=65)
                for jc in range(n_jc):
                    for h in range(HPC):
                        hs = slice(h * 64, (h + 1) * 64)
                        slot_base = jc * HPC * (n_tt + 1) + h * (n_tt + 1)
                        rline = rline_pool.tile([1, qw], F32, tag="rline")
                        qs = slice(jc * qw, (jc + 1) * qw)
                        av = pp.tile([65, qw], F32, tag="av", name="av")
                        pexps = {}
                        for kt in range(n_tt + 1):
                            if kt < n_tt:
                                st = pp.tile([128, qw], F32, tag="st",
                                             bufs=2, name="st")
                                for j in range(qw // 512):
                                    nc.tensor.matmul(
                                        st[:, j * 512:(j + 1) * 512],
                                        kT[hs, kt * 128:(kt + 1) * 128],
                                        qT[hs, jc * qw + j * 512:
                                           jc * qw + (j + 1) * 512],
                                        start=True, stop=True,
                                    )
                                # deferred normalizes for this batch's jc0
                                if jc == 1 and h == 0 and kt == 2:
                                    for hh in range(HPC):
                                        rline_t, at_t = pend_norm.pop((0, hh))
                                        emit_norm(rline_t, at_t, 0, hh)
                                    for tt in range(ntj):
                                        for half in range(2):
                                            d_queue.append(
                                                (bi, attnT, tt, half))
                                # pop D units: every other slot early, every
                                # slot in the last (jc, h) chunk
                                if (jc == n_jc - 1 and h == HPC - 1) \
                                        or (slot_base + kt) % 2 == 0:
                                    emit_d_unit()
                                pexp = pexp_pool.tile([128, qw], F16,
                                                      tag="pexp", name="pexp")
                                nc.scalar.activation(
                                    pexp[:, :], st[:, :], AF.Exp, scale=0.125
                                )
                                pexps[kt] = pexp
                            if kt > 0:
                                px = pexps.pop(kt - 1)
                                for j in range(qw // 512):
                                    nc.tensor.matmul(
                                        av[:, j * 512:(j + 1) * 512],
                                        vtv[:, kt - 1, h, :],
                                        px[:, j * 512:(j + 1) * 512],
                                        start=(kt == 1),
                                        stop=(kt == n_tt),
                                    )
                        nc.vector.tensor_copy(attnT[hs, qs], av[0:64, :])
                        nc.vector.tensor_copy(rline[0:1, :], av[64:65, :])
                        pend_norm[(jc, h)] = (rline, attnT)

                prev_tail = (bi, attnT, n_jc - 1)

            # ---- tail: finish last batch's final jc and flush its D ----
            bi_p, attnT_p, jc_p = prev_tail
            for h in range(HPC):
                rline_t, at_t = pend_norm.pop((jc_p, h))
                emit_norm(rline_t, at_t, jc_p, h)
            for tt in range(jc_p * ntj, (jc_p + 1) * ntj):
                for half in range(2):
                    d_queue.append((bi_p, attnT_p, tt, half))
            flip = False
            while d_queue:
                emit_d_unit(use_act=flip)
                flip = not flip

    nc.compile()
    return nc


def _get_nc(b, s):
    key = (b, s)
    if key not in _CACHE:
        _CACHE[key] = build_nc(b, s)
    return _CACHE[key]


def make_in_maps(x, w_q, b_q, w_k, b_k, w_v, w_o):
    x16 = np.ascontiguousarray(
        np.asarray(x, dtype=np.float16).transpose(0, 2, 1))
    wq16 = np.asarray(w_q, dtype=np.float16)
    wk16 = np.asarray(w_k, dtype=np.float16)
    wv16 = np.asarray(w_v, dtype=np.float16)
    wo16 = np.asarray(w_o, dtype=np.float16)
    in_maps = []
    for i in range(NCORES):
        cs = slice(i * DHC, (i + 1) * DHC)
        in_maps.append({
            "x": x16,
            "wq": np.ascontiguousarray(wq16[:, cs]),
            "wk": np.ascontiguousarray(wk16[:, cs]),
            "wv": np.ascontiguousarray(wv16[:, cs]),
            "bq": np.ascontiguousarray(b_q[cs, None], dtype=np.float32),
            "bk": np.ascontiguousarray(b_k[cs, None], dtype=np.float32),
            "wo": np.ascontiguousarray(wo16[cs, :]),
        })
    return in_maps


def kernel(x, w_q, b_q, w_k, b_k, w_v, b_v, w_o, b_o, _trace=False):
    x = np.asarray(x, dtype=np.float32)
    nc = _get_nc(x.shape[0], x.shape[1])
    in_maps = make_in_maps(x, w_q, b_q, w_k, b_k, w_v, w_o)
    kw = {}
    if _trace:
        import tempfile
        kw = dict(trace=True, trace_cores=list(range(NCORES)),
                  tmpdir=tempfile.mkdtemp(prefix="mha_trace_"))
    res = bass_utils.run_bass_kernel_spmd(
        nc, in_maps, core_ids=list(range(NCORES)), **kw
    )
    out = np.zeros(x.shape, dtype=np.float32)
    for i in range(NCORES):
        out += np.asarray(res.results[i]["out"], dtype=np.float32)
    out += np.asarray(b_o, dtype=np.float32)[None, None, :]
    out += (np.asarray(b_v, dtype=np.float32)
            @ np.asarray(w_o, dtype=np.float32))[None, None, :]
    if _trace:
        return out, res
    return out
